# revision 15
# baseline (speedup 1.0000x reference)
"""Trainium2 Bass kernel for the FLF (lookup-free) quantizer.

Math: the frozen codebook is every 16-bit sign pattern mapped {0,1}->{-1,+1},
and ||c||^2 == 16 for all codes, so

    argmin_k ||z - c_k||^2  ==  argmax_k <z, c_k>

decomposes per dimension: the winning code has c_j = +1 iff z_j > 0 (ties ->
-1, matching argmin's first-occurrence rule, since bit=0 lowers the index).
Hence with q = sign(z) in {-1,+1}:

    idx = (sum_j q_j * 2^(15-j) + 65535) / 2,   out = q @ W_out + b_out

which removes the [tokens, 65536] distance sweep entirely. The idx expression
is exact (products are signed powers of two, partial sums are integers
< 2^17, S + 65535 is even), even with bf16 matmul operands, since +-1 and 2^k
are exactly representable in bf16 and PSUM accumulates fp32. q is computed in
one DVE op via the fp32 bit trick (z & 0x80000000) | 0x3f800000 = +-1.0f
(z is never exactly 0 for gaussian inputs; z==+0.0 would give +1 vs the
reference's tie-break -1, a measure-zero case).

Sharding: data-parallel over the batch dim -- core b handles x[b] ([256, 128]
tokens), projections/codebook constants replicated. No cross-core comms.

Written in raw bass (explicit per-engine programs + semaphores, no Block):
this walrus build encodes at most ONE embedded sync-wait per compute
instruction, which Tile's auto-generated multi-sem waits violate; raw bass
emits standalone wait_ge ops instead.

Per-core pipeline (z kept transposed so the code dim e=16 sits on partitions):
    zT   [16,256]  = W_in.T @ xT (+ b_in ⊗ 1)   (PE fp32, K=128 [then K=1])
    q    [16,256]  = signbit(zT) | 1.0f         (DVE bitwise, one op)
    qb   [16,256]  = bf16(q)                    (DVE)
    outT [128,256] = W_out.T @ q (+ b_out ⊗ 1)  (PE fp32, K=16 [then K=1])
    S    [ 1,256]  = pow2.T @ qb                (PE bf16 1-pass, K=16)
    idx  [ 1,256]  = int32((S + 65535) * 0.5)   (DVE, exact)
The x shard loads as two halves on the two HWDGE engines (SP and ACT) with
mm1 split to match, so the first half's matmul overlaps the second half's
transfer; consts load second on ACT. outT stores via SP, idx via ACT. The b_in/b_out rank-1 accumulation matmuls
(bias ⊗ ones into PSUM; identical rounding to the reference's post-add) are
only emitted when the biases are nonzero. Host transposes x shards in / out
shards back (layout only).
"""

import os

import numpy as np

import concourse.bass as bass
from concourse import mybir
from concourse.bass_utils import run_bass_kernel_spmd

N_CORES = 8
B, T, D = 8, 256, 128  # x shape
DC = 16                # codebook dims

SIGN_MASK = -0x80000000   # int32 bit pattern 0x80000000
ONE_BITS = 0x3F800000     # fp32 1.0

# Packed input buffers (x halves split across two HWDGE engines):
#   A1 [128, 144] = [ W_in (16) | xT[:, 0:128] ]     via SP
#   A2 [128, 128] = [ xT[:, 128:256] ]               via ACT (first)
#   C  [ 16, CW]  = [ W_out (128) | pow2 bf16 col (1) | b_out row (128)
#                     | b_in row (16) | ones row (256) ]  via ACT (second)
TH = T // 2
A1W = DC + TH
C_W_OUT = 0
C_POW2 = D
C_BOUT = D + 1
C_BIN = C_BOUT + D
C_ONES = C_BIN + DC
CW_BIAS = C_ONES + T
CW_NOBIAS = D + 1

FINAL_WAIT = os.environ.get("BASS_FINAL_WAIT", "0") == "1"

_nc_cache = {}


def _build_bass(with_bias):
    f32 = mybir.dt.float32
    bf16 = mybir.dt.bfloat16
    i32 = mybir.dt.int32
    cw = CW_BIAS if with_bias else CW_NOBIAS
    nc = bass.Bass()

    a1_d = nc.dram_tensor("a1", [D, A1W], f32, kind="ExternalInput")
    a2_d = nc.dram_tensor("a2", [D, TH], f32, kind="ExternalInput")
    c_d = nc.dram_tensor("c", [DC, cw], f32, kind="ExternalInput")
    outT_d = nc.dram_tensor("outT", [D, T], f32, kind="ExternalOutput")
    idx_d = nc.dram_tensor("idx", [1, T], i32, kind="ExternalOutput")

    with (
        nc.sbuf_tensor("a_sb", [D, DC + T], f32) as a_sb,
        nc.sbuf_tensor("c_sb", [DC, cw], f32) as c_sb,
        nc.sbuf_tensor("q_sb", [DC, T], f32) as q_sb,
        nc.sbuf_tensor("qb_sb", [DC, T], bf16) as qb_sb,
        nc.sbuf_tensor("idx_sb", [1, T], i32) as idx_sb,
        nc.sbuf_tensor("outT_sb", [D, T], f32) as outT_sb,
        nc.psum_tensor("zT_ps", [DC, T], f32) as zT_ps,
        nc.psum_tensor("s_ps", [1, T], f32) as s_ps,
        nc.psum_tensor("outT_ps", [D, T], f32) as outT_ps,
        nc.semaphore("dma_a1") as dma_a1,
        nc.semaphore("dma_a2") as dma_a2,
        nc.semaphore("dma_c") as dma_c,
        nc.semaphore("pe_sem") as pe_sem,
        nc.semaphore("dve_sem") as dve_sem,
        nc.semaphore("dma_out") as dma_out,
    ):
        w_in = a_sb[:, 0:DC]
        xT0 = a_sb[:, DC:DC + TH]
        xT1 = a_sb[:, DC + TH:DC + T]
        xT = a_sb[:, DC:DC + T]
        w_out = c_sb[:, C_W_OUT:C_W_OUT + D]
        pow2_bf = c_sb[:, C_POW2:C_POW2 + 1].bitcast(bf16)[:, 0:1]

        # --- loads (parallel descriptor gen on two HWDGE engines) ---
        nc.sync.dma_start(out=a_sb[:, 0:A1W], in_=a1_d[:, :]).then_inc(
            dma_a1, 16)
        nc.scalar.dma_start(out=a_sb[:, A1W:A1W + TH], in_=a2_d[:, :]).then_inc(
            dma_a2, 16)
        nc.scalar.dma_start(out=c_sb[:, :], in_=c_d[:, :]).then_inc(dma_c, 16)

        # --- PE: zT (two half-N chunks pipelined behind the loads, or
        #     full-width + rank-1 bias accumulation when biases are set) ---
        nc.tensor.wait_ge(dma_a1, 16)
        if with_bias:
            bin_row = c_sb[0:1, C_BIN:C_BIN + DC]
            bout_row = c_sb[0:1, C_BOUT:C_BOUT + D]
            ones_row = c_sb[0:1, C_ONES:C_ONES + T]
            nc.tensor.wait_ge(dma_a2, 16)
            nc.tensor.matmul(zT_ps[:, :], w_in, xT, start=True, stop=False)
            nc.tensor.wait_ge(dma_c, 16)
            nc.tensor.matmul(zT_ps[:, :], bin_row, ones_row,
                             start=False, stop=True).then_inc(pe_sem, 1)
        else:
            nc.tensor.matmul(zT_ps[:, 0:TH], w_in, xT0, start=True, stop=True)
            nc.tensor.wait_ge(dma_a2, 16)
            nc.tensor.matmul(zT_ps[:, TH:T], w_in, xT1,
                             start=True, stop=True).then_inc(pe_sem, 1)
            nc.tensor.wait_ge(dma_c, 16)

        # --- DVE: q from sign bits (one op), then bf16 copy ---
        nc.vector.wait_ge(pe_sem, 1)
        nc.vector.tensor_scalar(
            out=q_sb[:, :].bitcast(i32), in0=zT_ps[:, :].bitcast(i32),
            scalar1=SIGN_MASK, scalar2=ONE_BITS,
            op0=mybir.AluOpType.bitwise_and, op1=mybir.AluOpType.bitwise_or,
        ).then_inc(dve_sem, 1)
        nc.vector.wait_ge(dve_sem, 1)
        nc.vector.tensor_copy(out=qb_sb[:, :], in_=q_sb[:, :]).then_inc(
            dve_sem, 1)

        # --- PE: outT (critical path) then S ---
        nc.tensor.wait_ge(dve_sem, 1)
        if with_bias:
            nc.tensor.matmul(outT_ps[:, :], w_out, q_sb[:, :],
                             start=True, stop=False)
            nc.tensor.matmul(outT_ps[:, :], bout_row, ones_row,
                             start=False, stop=True).then_inc(pe_sem, 1)
        else:
            nc.tensor.matmul(outT_ps[:, :], w_out, q_sb[:, :],
                             start=True, stop=True).then_inc(pe_sem, 1)
        nc.tensor.wait_ge(dve_sem, 2)
        nc.tensor.matmul(s_ps[:, :], pow2_bf, qb_sb[:, :],
                         start=True, stop=True).then_inc(pe_sem, 1)

        # --- DVE: copy outT, compute idx ---
        nc.vector.wait_ge(pe_sem, 2)
        nc.vector.tensor_copy(
            out=outT_sb[:, :], in_=outT_ps[:, :]).then_inc(dve_sem, 1)
        nc.vector.wait_ge(pe_sem, 3)
        nc.vector.tensor_scalar(
            out=idx_sb[:, :], in0=s_ps[:, :], scalar1=65535.0, scalar2=0.5,
            op0=mybir.AluOpType.add, op1=mybir.AluOpType.mult,
        ).then_inc(dve_sem, 1)

        # --- stores ---
        nc.sync.wait_ge(dve_sem, 3)
        nc.sync.dma_start(out=outT_d[:, :], in_=outT_sb[:, :]).then_inc(
            dma_out, 16)
        nc.scalar.wait_ge(dve_sem, 4)
        nc.scalar.dma_start(out=idx_d[:, :], in_=idx_sb[:, :]).then_inc(
            dma_out, 16)
        if FINAL_WAIT:
            nc.sync.wait_ge(dma_out, 32)

    return nc


def _get_nc(with_bias):
    if with_bias not in _nc_cache:
        _nc_cache[with_bias] = _build_bass(with_bias)
    return _nc_cache[with_bias]


def _pack_inputs(x, W_in, b_in, W_out, b_out, with_bias):
    cw = CW_BIAS if with_bias else CW_NOBIAS
    c_buf = np.zeros((DC, cw), dtype=np.float32)
    c_buf[:, C_W_OUT:C_W_OUT + D] = W_out
    pow2 = 2.0 ** np.arange(DC - 1, -1, -1, dtype=np.float32)
    # bf16 bit pattern of 2^k in the low 2 bytes of the fp32 slot
    c_buf.view(np.uint32)[:, C_POW2] = pow2.view(np.uint32) >> 16
    if with_bias:
        c_buf[0, C_BOUT:C_BOUT + D] = b_out
        c_buf[0, C_BIN:C_BIN + DC] = b_in
        c_buf[0, C_ONES:C_ONES + T] = 1.0

    in_maps = []
    for c in range(N_CORES):
        xt = x[c].T
        a1_buf = np.empty((D, A1W), dtype=np.float32)
        a1_buf[:, 0:DC] = W_in
        a1_buf[:, DC:DC + TH] = xt[:, 0:TH]
        a2_buf = np.ascontiguousarray(xt[:, TH:T])
        in_maps.append({"a1": a1_buf, "a2": a2_buf, "c": c_buf})
    return in_maps


def kernel(x, W_in, b_in, W_out, b_out, **run_kwargs):
    x = np.asarray(x, dtype=np.float32)
    W_in = np.asarray(W_in, dtype=np.float32)
    W_out = np.asarray(W_out, dtype=np.float32)
    b_in = np.asarray(b_in, dtype=np.float32)
    b_out = np.asarray(b_out, dtype=np.float32)

    with_bias = bool(np.any(b_in) or np.any(b_out))
    in_maps = _pack_inputs(x, W_in, b_in, W_out, b_out, with_bias)
    nc = _get_nc(with_bias)
    res = run_bass_kernel_spmd(nc, in_maps, core_ids=list(range(N_CORES)),
                               **run_kwargs)

    out = np.stack([res.results[c]["outT"].T for c in range(N_CORES)])
    idx = np.stack([res.results[c]["idx"].reshape(T) for c in range(N_CORES)])
    out = np.ascontiguousarray(out)
    idx = np.ascontiguousarray(idx.astype(np.int32))
    if run_kwargs:
        return (out, idx), res
    return out, idx


# revision 22
# speedup vs baseline: 1.0325x; 1.0325x over previous
"""Trainium2 Bass kernel for the FLF (lookup-free) quantizer.

Math: the frozen codebook is every 16-bit sign pattern mapped {0,1}->{-1,+1},
and ||c||^2 == 16 for all codes, so

    argmin_k ||z - c_k||^2  ==  argmax_k <z, c_k>

decomposes per dimension: the winning code has c_j = +1 iff z_j > 0 (ties ->
-1, matching argmin's first-occurrence rule, since bit=0 lowers the index).
Hence with q = sign(z) in {-1,+1}:

    idx = (sum_j q_j * 2^(15-j) + 65535) / 2,   out = q @ W_out + b_out

which removes the [tokens, 65536] distance sweep entirely. The idx expression
is exact (products are signed powers of two, partial sums are integers
< 2^17, S + 65535 is even), even with bf16 matmul operands, since +-1 and 2^k
are exactly representable in bf16 and PSUM accumulates fp32. q is computed in
one DVE op via the fp32 bit trick (z & 0x80000000) | 0x3f800000 = +-1.0f
(z is never exactly 0 for gaussian inputs; z==+0.0 would give +1 vs the
reference's tie-break -1, a measure-zero case).

Sharding: data-parallel over the batch dim -- core b handles x[b] ([256, 128]
tokens), projections/codebook constants replicated. No cross-core comms.

Written in raw bass (explicit per-engine programs + semaphores, no Block):
this walrus build encodes at most ONE embedded sync-wait per compute
instruction, which Tile's auto-generated multi-sem waits violate; raw bass
emits standalone wait_ge ops instead.

Per-core pipeline (z kept transposed so the code dim e=16 sits on partitions):
    zT   [16,256]  = W_in.T @ xT (+ b_in ⊗ 1)   (PE fp32, K=128 [then K=1])
    q    [16,256]  = signbit(zT) | 1.0f         (DVE bitwise, one op)
    qb   [16,256]  = bf16(q)                    (DVE)
    outT [128,256] = W_out.T @ q (+ b_out ⊗ 1)  (PE fp32, K=16 [then K=1])
    S    [ 1,256]  = pow2.T @ qb                (PE bf16 1-pass, K=16)
    idx  [ 1,256]  = int32((S + 65535) * 0.5)   (DVE, exact)
The x shard loads as two halves on the two HWDGE engines (SP and ACT) with
mm1 split to match, so the first half's matmul overlaps the second half's
transfer; consts load second on ACT. outT stores via SP, idx via ACT. The b_in/b_out rank-1 accumulation matmuls
(bias ⊗ ones into PSUM; identical rounding to the reference's post-add) are
only emitted when the biases are nonzero. Host transposes x shards in / out
shards back (layout only).
"""

import os

import numpy as np

import concourse.bass as bass
from concourse import mybir
from concourse.bass_utils import run_bass_kernel_spmd

N_CORES = 8
B, T, D = 8, 256, 128  # x shape
DC = 16                # codebook dims

SIGN_MASK = -0x80000000   # int32 bit pattern 0x80000000
ONE_BITS = 0x3F800000     # fp32 1.0

# Packed input buffers (x halves split across two HWDGE engines):
#   A1 [128, 144] = [ W_in (16) | xT[:, 0:128] ]     via SP
#   A2 [128, 128] = [ xT[:, 128:256] ]               via ACT (first)
#   C  [ 16, CW]  = [ W_out (128) | pow2 bf16 col (1) | b_out row (128)
#                     | b_in row (16) | ones row (256) ]  via ACT (second)
TH = T // 2
A1W = DC + TH
C_W_OUT = 0
C_POW2 = D
C_BOUT = D + 1
C_BIN = C_BOUT + D
C_ONES = C_BIN + DC
CW_BIAS = C_ONES + T
CW_NOBIAS = D + 1

FINAL_WAIT = os.environ.get("BASS_FINAL_WAIT", "0") == "1"

_nc_cache = {}


def _build_bass(with_bias):
    f32 = mybir.dt.float32
    bf16 = mybir.dt.bfloat16
    i32 = mybir.dt.int32
    cw = CW_BIAS if with_bias else CW_NOBIAS
    nc = bass.Bass()

    a1_d = nc.dram_tensor("a1", [D, A1W], f32, kind="ExternalInput")
    a2_d = nc.dram_tensor("a2", [D, TH], f32, kind="ExternalInput")
    c_d = nc.dram_tensor("c", [DC, cw], f32, kind="ExternalInput")
    outT_d = nc.dram_tensor("outT", [D, T], f32, kind="ExternalOutput")
    idx_d = nc.dram_tensor("idx", [1, T], i32, kind="ExternalOutput")

    with (
        nc.sbuf_tensor("a_sb", [D, DC + T], f32) as a_sb,
        nc.sbuf_tensor("c_sb", [DC, cw], f32) as c_sb,
        nc.sbuf_tensor("q_sb", [DC, T], f32) as q_sb,
        nc.sbuf_tensor("qb_sb", [DC, T], bf16) as qb_sb,
        nc.sbuf_tensor("idx_sb", [1, T], i32) as idx_sb,
        nc.sbuf_tensor("outT_sb", [D, T], f32) as outT_sb,
        nc.psum_tensor("zT_ps", [DC, T], f32) as zT_ps,
        nc.psum_tensor("zT0_ps", [DC, TH], f32) as zT0_ps,
        nc.psum_tensor("zT1_ps", [DC, TH], f32) as zT1_ps,
        nc.psum_tensor("s_ps", [1, T], f32) as s_ps,
        nc.psum_tensor("outT_ps", [D, T], f32) as outT_ps,
        nc.semaphore("dma_a1") as dma_a1,
        nc.semaphore("dma_a2") as dma_a2,
        nc.semaphore("dma_c") as dma_c,
        nc.semaphore("pe_sem") as pe_sem,
        nc.semaphore("dve_sem") as dve_sem,
        nc.semaphore("dma_out") as dma_out,
    ):
        w_in = a_sb[:, 0:DC]
        xT0 = a_sb[:, DC:DC + TH]
        xT1 = a_sb[:, DC + TH:DC + T]
        xT = a_sb[:, DC:DC + T]
        w_out = c_sb[:, C_W_OUT:C_W_OUT + D]
        pow2_bf = c_sb[:, C_POW2:C_POW2 + 1].bitcast(bf16)[:, 0:1]

        # --- loads (parallel descriptor gen on two HWDGE engines) ---
        nc.sync.dma_start(out=a_sb[:, 0:A1W], in_=a1_d[:, :]).then_inc(
            dma_a1, 16)
        nc.scalar.dma_start(out=a_sb[:, A1W:A1W + TH], in_=a2_d[:, :]).then_inc(
            dma_a2, 16)
        nc.scalar.dma_start(out=c_sb[:, :], in_=c_d[:, :]).then_inc(dma_c, 16)

        # --- PE: zT (two half-N chunks pipelined behind the loads, or
        #     full-width + rank-1 bias accumulation when biases are set) ---
        nc.tensor.wait_ge(dma_a1, 16)
        if with_bias:
            bin_row = c_sb[0:1, C_BIN:C_BIN + DC]
            bout_row = c_sb[0:1, C_BOUT:C_BOUT + D]
            ones_row = c_sb[0:1, C_ONES:C_ONES + T]
            nc.tensor.wait_ge(dma_a2, 16)
            nc.tensor.matmul(zT_ps[:, :], w_in, xT, start=True, stop=False)
            nc.tensor.wait_ge(dma_c, 16)
            nc.tensor.matmul(zT_ps[:, :], bin_row, ones_row,
                             start=False, stop=True).then_inc(pe_sem, 1)
        else:
            # separate PSUM banks per half: the DVE reads half 0 while the
            # PE still writes half 1 (same-bank PE-write + DVE-read is a
            # fatal hazard on TRN2)
            nc.tensor.matmul(zT0_ps[:, :], w_in, xT0,
                             start=True, stop=True).then_inc(pe_sem, 1)
            nc.tensor.wait_ge(dma_a2, 16)
            nc.tensor.matmul(zT1_ps[:, :], w_in, xT1,
                             start=True, stop=True).then_inc(pe_sem, 1)
            nc.tensor.wait_ge(dma_c, 16)

        # --- DVE: q from sign bits, then bf16 copy. In the no-bias path q
        # is computed per token half, pipelined behind the two mm1 chunks, so
        # mm3 starts as soon as the second half lands. ---
        def _q(dst, src):
            return nc.vector.tensor_scalar(
                out=dst.bitcast(i32), in0=src.bitcast(i32),
                scalar1=SIGN_MASK, scalar2=ONE_BITS,
                op0=mybir.AluOpType.bitwise_and,
                op1=mybir.AluOpType.bitwise_or,
            )

        if with_bias:
            nc.vector.wait_ge(pe_sem, 1)
            _q(q_sb[:, :], zT_ps[:, :]).then_inc(dve_sem, 1)
        else:
            nc.vector.wait_ge(pe_sem, 1)
            _q(q_sb[:, 0:TH], zT0_ps[:, :])
            nc.vector.wait_ge(pe_sem, 2)
            _q(q_sb[:, TH:T], zT1_ps[:, :]).then_inc(dve_sem, 1)
        nc.vector.wait_ge(dve_sem, 1)
        nc.vector.tensor_copy(out=qb_sb[:, :], in_=q_sb[:, :]).then_inc(
            dve_sem, 1)

        # --- PE: outT (critical path) then S ---
        pe_mm1 = 1 if with_bias else 2
        nc.tensor.wait_ge(dve_sem, 1)
        if with_bias:
            nc.tensor.matmul(outT_ps[:, :], w_out, q_sb[:, :],
                             start=True, stop=False)
            nc.tensor.matmul(outT_ps[:, :], bout_row, ones_row,
                             start=False, stop=True).then_inc(pe_sem, 1)
        else:
            nc.tensor.matmul(outT_ps[:, :], w_out, q_sb[:, :],
                             start=True, stop=True).then_inc(pe_sem, 1)
        nc.tensor.wait_ge(dve_sem, 2)
        nc.tensor.matmul(s_ps[:, :], pow2_bf, qb_sb[:, :],
                         start=True, stop=True).then_inc(pe_sem, 1)

        # --- DVE: copy outT, compute idx ---
        nc.vector.wait_ge(pe_sem, pe_mm1 + 1)
        nc.vector.tensor_copy(
            out=outT_sb[:, :], in_=outT_ps[:, :]).then_inc(dve_sem, 1)
        nc.vector.wait_ge(pe_sem, pe_mm1 + 2)
        nc.vector.tensor_scalar(
            out=idx_sb[:, :], in0=s_ps[:, :], scalar1=65535.0, scalar2=0.5,
            op0=mybir.AluOpType.add, op1=mybir.AluOpType.mult,
        ).then_inc(dve_sem, 1)

        # --- stores ---
        nc.sync.wait_ge(dve_sem, 3)
        nc.sync.dma_start(out=outT_d[:, :], in_=outT_sb[:, :]).then_inc(
            dma_out, 16)
        nc.scalar.wait_ge(dve_sem, 4)
        nc.scalar.dma_start(out=idx_d[:, :], in_=idx_sb[:, :]).then_inc(
            dma_out, 16)
        if FINAL_WAIT:
            nc.sync.wait_ge(dma_out, 32)

    return nc


def _get_nc(with_bias):
    if with_bias not in _nc_cache:
        _nc_cache[with_bias] = _build_bass(with_bias)
    return _nc_cache[with_bias]


def _pack_inputs(x, W_in, b_in, W_out, b_out, with_bias):
    cw = CW_BIAS if with_bias else CW_NOBIAS
    c_buf = np.zeros((DC, cw), dtype=np.float32)
    c_buf[:, C_W_OUT:C_W_OUT + D] = W_out
    pow2 = 2.0 ** np.arange(DC - 1, -1, -1, dtype=np.float32)
    # bf16 bit pattern of 2^k in the low 2 bytes of the fp32 slot
    c_buf.view(np.uint32)[:, C_POW2] = pow2.view(np.uint32) >> 16
    if with_bias:
        c_buf[0, C_BOUT:C_BOUT + D] = b_out
        c_buf[0, C_BIN:C_BIN + DC] = b_in
        c_buf[0, C_ONES:C_ONES + T] = 1.0

    in_maps = []
    for c in range(N_CORES):
        xt = x[c].T
        a1_buf = np.empty((D, A1W), dtype=np.float32)
        a1_buf[:, 0:DC] = W_in
        a1_buf[:, DC:DC + TH] = xt[:, 0:TH]
        a2_buf = np.ascontiguousarray(xt[:, TH:T])
        in_maps.append({"a1": a1_buf, "a2": a2_buf, "c": c_buf})
    return in_maps


def kernel(x, W_in, b_in, W_out, b_out, **run_kwargs):
    x = np.asarray(x, dtype=np.float32)
    W_in = np.asarray(W_in, dtype=np.float32)
    W_out = np.asarray(W_out, dtype=np.float32)
    b_in = np.asarray(b_in, dtype=np.float32)
    b_out = np.asarray(b_out, dtype=np.float32)

    with_bias = bool(np.any(b_in) or np.any(b_out))
    in_maps = _pack_inputs(x, W_in, b_in, W_out, b_out, with_bias)
    nc = _get_nc(with_bias)
    res = run_bass_kernel_spmd(nc, in_maps, core_ids=list(range(N_CORES)),
                               **run_kwargs)

    out = np.stack([res.results[c]["outT"].T for c in range(N_CORES)])
    idx = np.stack([res.results[c]["idx"].reshape(T) for c in range(N_CORES)])
    out = np.ascontiguousarray(out)
    idx = np.ascontiguousarray(idx.astype(np.int32))
    if run_kwargs:
        return (out, idx), res
    return out, idx


# revision 25
# speedup vs baseline: 1.0434x; 1.0105x over previous
"""Trainium2 Bass kernel for the FLF (lookup-free) quantizer.

Math: the frozen codebook is every 16-bit sign pattern mapped {0,1}->{-1,+1},
and ||c||^2 == 16 for all codes, so

    argmin_k ||z - c_k||^2  ==  argmax_k <z, c_k>

decomposes per dimension: the winning code has c_j = +1 iff z_j > 0 (ties ->
-1, matching argmin's first-occurrence rule, since bit=0 lowers the index).
Hence with q = sign(z) in {-1,+1}:

    idx = (sum_j q_j * 2^(15-j) + 65535) / 2,   out = q @ W_out + b_out

which removes the [tokens, 65536] distance sweep entirely. The idx expression
is exact (products are signed powers of two, partial sums are integers
< 2^17, S + 65535 is even), even with bf16 matmul operands, since +-1 and 2^k
are exactly representable in bf16 and PSUM accumulates fp32. q is computed in
one DVE op via the fp32 bit trick (z & 0x80000000) | 0x3f800000 = +-1.0f
(z is never exactly 0 for gaussian inputs; z==+0.0 would give +1 vs the
reference's tie-break -1, a measure-zero case).

Sharding: data-parallel over the batch dim -- core b handles x[b] ([256, 128]
tokens), projections/codebook constants replicated. No cross-core comms.

Written in raw bass (explicit per-engine programs + semaphores, no Block):
this walrus build encodes at most ONE embedded sync-wait per compute
instruction, which Tile's auto-generated multi-sem waits violate; raw bass
emits standalone wait_ge ops instead.

Per-core pipeline (z kept transposed so the code dim e=16 sits on partitions):
    zT   [16,256]  = W_in.T @ xT (+ b_in ⊗ 1)   (PE fp32, K=128 [then K=1])
    q    [16,256]  = signbit(zT) | 1.0f         (DVE bitwise, one op)
    qb   [16,256]  = bf16(q)                    (DVE)
    outT [128,256] = W_out.T @ q (+ b_out ⊗ 1)  (PE fp32, K=16 [then K=1])
    S    [ 1,256]  = pow2.T @ qb                (PE bf16 1-pass, K=16)
    idx  [ 1,256]  = int32((S + 65535) * 0.5)   (DVE, exact)
The x shard loads as two halves on the two HWDGE engines (SP and ACT) with
mm1 split to match, so the first half's matmul overlaps the second half's
transfer; consts load second on ACT. outT stores via SP, idx via ACT. The b_in/b_out rank-1 accumulation matmuls
(bias ⊗ ones into PSUM; identical rounding to the reference's post-add) are
only emitted when the biases are nonzero. Host transposes x shards in / out
shards back (layout only).
"""

import os

import numpy as np

import concourse.bass as bass
from concourse import mybir
from concourse.bass_utils import run_bass_kernel_spmd

N_CORES = 8
B, T, D = 8, 256, 128  # x shape
DC = 16                # codebook dims

SIGN_MASK = -0x80000000   # int32 bit pattern 0x80000000
ONE_BITS = 0x3F800000     # fp32 1.0

# Packed input buffers (x halves split across two HWDGE engines):
#   A1 [128, 144] = [ W_in (16) | xT[:, 0:128] ]     via SP
#   A2 [128, 128] = [ xT[:, 128:256] ]               via ACT (first)
#   C  [ 16, CW]  = [ W_out (128) | pow2 bf16 col (1) | b_out row (128)
#                     | b_in row (16) | ones row (256) ]  via ACT (second)
TH = T // 2
A1W = DC + TH
C_W_OUT = 0
C_POW2 = D
C_BOUT = D + 1
C_BIN = C_BOUT + D
C_ONES = C_BIN + DC
CW_BIAS = C_ONES + T
CW_NOBIAS = D + 1

FINAL_WAIT = os.environ.get("BASS_FINAL_WAIT", "0") == "1"

_nc_cache = {}


def _build_bass(with_bias):
    f32 = mybir.dt.float32
    bf16 = mybir.dt.bfloat16
    i32 = mybir.dt.int32
    cw = CW_BIAS if with_bias else CW_NOBIAS
    nc = bass.Bass()

    a1_d = nc.dram_tensor("a1", [D, A1W], f32, kind="ExternalInput")
    a2_d = nc.dram_tensor("a2", [D, TH], f32, kind="ExternalInput")
    c_d = nc.dram_tensor("c", [DC, cw], f32, kind="ExternalInput")
    outT_d = nc.dram_tensor("outT", [D, T], f32, kind="ExternalOutput")
    idx_d = nc.dram_tensor("idx", [1, T], i32, kind="ExternalOutput")

    with (
        nc.sbuf_tensor("a_sb", [D, DC + T], f32) as a_sb,
        nc.sbuf_tensor("c_sb", [DC, cw], f32) as c_sb,
        nc.sbuf_tensor("q_sb", [DC, T], f32) as q_sb,
        nc.sbuf_tensor("qb_sb", [DC, T], bf16) as qb_sb,
        nc.sbuf_tensor("idx_sb", [1, T], i32) as idx_sb,
        nc.sbuf_tensor("outT_sb", [D, T], f32) as outT_sb,
        nc.psum_tensor("zT_ps", [DC, T], f32) as zT_ps,
        nc.psum_tensor("zT0_ps", [DC, TH], f32) as zT0_ps,
        nc.psum_tensor("zT1_ps", [DC, TH], f32) as zT1_ps,
        nc.psum_tensor("s_ps", [1, T], f32) as s_ps,
        nc.psum_tensor("outT_ps", [D, T], f32) as outT_ps,
        nc.semaphore("dma_a1") as dma_a1,
        nc.semaphore("dma_a2") as dma_a2,
        nc.semaphore("dma_c") as dma_c,
        nc.semaphore("pe_sem") as pe_sem,
        nc.semaphore("dve_sem") as dve_sem,
        nc.semaphore("dma_out") as dma_out,
    ):
        w_in = a_sb[:, 0:DC]
        xT0 = a_sb[:, DC:DC + TH]
        xT1 = a_sb[:, DC + TH:DC + T]
        xT = a_sb[:, DC:DC + T]
        w_out = c_sb[:, C_W_OUT:C_W_OUT + D]
        pow2_bf = c_sb[:, C_POW2:C_POW2 + 1].bitcast(bf16)[:, 0:1]

        # --- loads (parallel descriptor gen on two HWDGE engines) ---
        nc.sync.dma_start(out=a_sb[:, 0:A1W], in_=a1_d[:, :]).then_inc(
            dma_a1, 16)
        nc.scalar.dma_start(out=a_sb[:, A1W:A1W + TH], in_=a2_d[:, :]).then_inc(
            dma_a2, 16)
        nc.scalar.dma_start(out=c_sb[:, :], in_=c_d[:, :]).then_inc(dma_c, 16)

        # --- PE: zT (two half-N chunks pipelined behind the loads, or
        #     full-width + rank-1 bias accumulation when biases are set) ---
        nc.tensor.wait_ge(dma_a1, 16)
        if with_bias:
            bin_row = c_sb[0:1, C_BIN:C_BIN + DC]
            bout_row = c_sb[0:1, C_BOUT:C_BOUT + D]
            ones_row = c_sb[0:1, C_ONES:C_ONES + T]
            nc.tensor.wait_ge(dma_a2, 16)
            nc.tensor.matmul(zT_ps[:, :], w_in, xT, start=True, stop=False)
            nc.tensor.wait_ge(dma_c, 16)
            nc.tensor.matmul(zT_ps[:, :], bin_row, ones_row,
                             start=False, stop=True).then_inc(pe_sem, 1)
        else:
            # separate PSUM banks per half: the DVE reads half 0 while the
            # PE still writes half 1 (same-bank PE-write + DVE-read is a
            # fatal hazard on TRN2)
            nc.tensor.matmul(zT0_ps[:, :], w_in, xT0,
                             start=True, stop=True).then_inc(pe_sem, 1)
            nc.tensor.wait_ge(dma_a2, 16)
            nc.tensor.matmul(zT1_ps[:, :], w_in, xT1,
                             start=True, stop=True).then_inc(pe_sem, 1)
            nc.tensor.wait_ge(dma_c, 16)

        # --- DVE: q from sign bits, then bf16 copy. In the no-bias path q
        # is computed per token half, pipelined behind the two mm1 chunks, so
        # mm3 starts as soon as the second half lands. ---
        def _q(dst, src):
            return nc.vector.tensor_scalar(
                out=dst.bitcast(i32), in0=src.bitcast(i32),
                scalar1=SIGN_MASK, scalar2=ONE_BITS,
                op0=mybir.AluOpType.bitwise_and,
                op1=mybir.AluOpType.bitwise_or,
            )

        if with_bias:
            nc.vector.wait_ge(pe_sem, 1)
            _q(q_sb[:, :], zT_ps[:, :]).then_inc(dve_sem, 1)
        else:
            nc.vector.wait_ge(pe_sem, 1)
            _q(q_sb[:, 0:TH], zT0_ps[:, :])
            nc.vector.wait_ge(pe_sem, 2)
            _q(q_sb[:, TH:T], zT1_ps[:, :]).then_inc(dve_sem, 1)
        nc.vector.wait_ge(dve_sem, 1)
        nc.vector.tensor_copy(out=qb_sb[:, :], in_=q_sb[:, :]).then_inc(
            dve_sem, 1)

        # --- PE: outT (critical path) then S ---
        pe_mm1 = 1 if with_bias else 2
        nc.tensor.wait_ge(dve_sem, 1)
        if with_bias:
            nc.tensor.matmul(outT_ps[:, :], w_out, q_sb[:, :],
                             start=True, stop=False)
            nc.tensor.matmul(outT_ps[:, :], bout_row, ones_row,
                             start=False, stop=True).then_inc(pe_sem, 1)
        else:
            nc.tensor.matmul(outT_ps[:, :], w_out, q_sb[:, :],
                             start=True, stop=True).then_inc(pe_sem, 1)
        nc.tensor.wait_ge(dve_sem, 2)
        nc.tensor.matmul(s_ps[:, :], pow2_bf, qb_sb[:, :],
                         start=True, stop=True).then_inc(pe_sem, 1)

        # --- DVE: copy outT, compute idx ---
        nc.vector.wait_ge(pe_sem, pe_mm1 + 1)
        nc.vector.tensor_copy(
            out=outT_sb[:, :], in_=outT_ps[:, :]).then_inc(dve_sem, 1)
        nc.vector.wait_ge(pe_sem, pe_mm1 + 2)
        nc.vector.tensor_scalar(
            out=idx_sb[:, :], in0=s_ps[:, :], scalar1=65535.0, scalar2=0.5,
            op0=mybir.AluOpType.add, op1=mybir.AluOpType.mult,
        ).then_inc(dve_sem, 1)

        # --- stores ---
        nc.sync.wait_ge(dve_sem, 3)
        nc.sync.dma_start(out=outT_d[:, :], in_=outT_sb[:, :]).then_inc(
            dma_out, 16)
        nc.scalar.wait_ge(dve_sem, 4)
        nc.scalar.dma_start(out=idx_d[:, :], in_=idx_sb[:, :]).then_inc(
            dma_out, 16)
        if FINAL_WAIT:
            nc.sync.wait_ge(dma_out, 32)

    return nc


def _get_nc(with_bias):
    if with_bias not in _nc_cache:
        _nc_cache[with_bias] = _build_bass(with_bias)
    return _nc_cache[with_bias]


def _pack_inputs(x, W_in, b_in, W_out, b_out, with_bias):
    cw = CW_BIAS if with_bias else CW_NOBIAS
    c_buf = np.zeros((DC, cw), dtype=np.float32)
    c_buf[:, C_W_OUT:C_W_OUT + D] = W_out
    pow2 = 2.0 ** np.arange(DC - 1, -1, -1, dtype=np.float32)
    # bf16 bit pattern of 2^k in the low 2 bytes of the fp32 slot
    c_buf.view(np.uint32)[:, C_POW2] = pow2.view(np.uint32) >> 16
    if with_bias:
        c_buf[0, C_BOUT:C_BOUT + D] = b_out
        c_buf[0, C_BIN:C_BIN + DC] = b_in
        c_buf[0, C_ONES:C_ONES + T] = 1.0

    in_maps = []
    for c in range(N_CORES):
        xt = x[c].T
        a1_buf = np.empty((D, A1W), dtype=np.float32)
        a1_buf[:, 0:DC] = W_in
        a1_buf[:, DC:DC + TH] = xt[:, 0:TH]
        a2_buf = np.ascontiguousarray(xt[:, TH:T])
        in_maps.append({"a1": a1_buf, "a2": a2_buf, "c": c_buf})
    return in_maps


def kernel(x, W_in, b_in, W_out, b_out, **run_kwargs):
    x = np.asarray(x, dtype=np.float32)
    W_in = np.asarray(W_in, dtype=np.float32)
    W_out = np.asarray(W_out, dtype=np.float32)
    b_in = np.asarray(b_in, dtype=np.float32)
    b_out = np.asarray(b_out, dtype=np.float32)

    with_bias = bool(np.any(b_in) or np.any(b_out))
    in_maps = _pack_inputs(x, W_in, b_in, W_out, b_out, with_bias)
    nc = _get_nc(with_bias)
    res = run_bass_kernel_spmd(nc, in_maps, core_ids=list(range(N_CORES)),
                               **run_kwargs)

    out = np.stack([res.results[c]["outT"].T for c in range(N_CORES)])
    idx = np.stack([res.results[c]["idx"].reshape(T) for c in range(N_CORES)])
    out = np.ascontiguousarray(out)
    idx = np.ascontiguousarray(idx.astype(np.int32))
    if run_kwargs:
        return (out, idx), res
    return out, idx
